# revision 11
# baseline (speedup 1.0000x reference)
"""Trainium2 Bass kernel for nn_MultiHeadCrossAttention_67963562492589.

Reference computation (B=16, S=1024, H=4, QD=128, KD=VD=256):
    tq = (query @ Wq + bq).view(B, H, 1024, 256)   # torch .view semantics!
    tk = (key   @ Wk + bk).view(B, H, 1024, 256)
    tv = (value @ Wv + bv).view(B, H, 1024, 256)
    scores   = tq @ tk^T          (no 1/sqrt(d) scaling)
    attn     = softmax(scores, -1)                  # [B,H,1024,1024] OUTPUT
    attended = attn @ tv  -> .view(B, 1024, 1024)
    out      = ((attended @ Wo + bo).mean(1)) @ W1 + b1   # [B,128] OUTPUT

Key algebraic facts used here:
  * The .view head-split means head h covers flat rows h*1024..h*1024+1023 of
    the [4096, 256] projected matrix; row r corresponds to original sequence
    position s = r//4 and feature-quarter j = r%4.  So head h attends over
    original positions s in [h*256, (h+1)*256).
  * mean-before-matmul: out depends on attn only through the per-(b,h)
    column sums of attn grouped by (row mod 4):
        R[j, k] = sum_{q = j mod 4} attn[q, k]          (mask matmul)
        Z[j, :] = sum_h sum_k R[j,k] * tv_h[k, :]       (tiny matmuls)
        out     = (Z.flatten()/1024) @ (Wo @ W1) + (bo @ W1 + b1)
    so `attended` is never materialized and Wo/W1 fold into one [1024,128]
    constant computed on the host.

Sharding: pure data parallel — batch 16 -> 2 per core across 8 cores.  Each
core holds the full (folded) weights, computes attn for its 2 batches plus
the [2,128] `out` rows.  All matmuls run as float32r (TF32-like, full PE
speed at N>=256), softmax exp on ScalarE (fp32, no max-subtraction needed:
|scores| < ~15 for any plausible input scale here).
"""

import numpy as np

B, S, H = 16, 1024, 4
QD, KD, VD = 128, 256, 256
N_CORES = 8
B_LOC = B // N_CORES  # 2 batches per core

_CACHE = {}


def _build_nc():
    import concourse.mybir as mybir
    import concourse.tile as tile
    from concourse import bacc
    from contextlib import ExitStack

    f32 = mybir.dt.float32
    f32r = mybir.dt.float32r
    AF = mybir.ActivationFunctionType

    nc = bacc.Bacc("TRN2", target_bir_lowering=False, debug=False,
                   num_devices=N_CORES)

    # ---- DRAM parameters -------------------------------------------------
    q_d = nc.dram_tensor("q", [B_LOC, S, QD], f32, kind="ExternalInput").ap()
    k_d = nc.dram_tensor("k", [B_LOC, S, KD], f32, kind="ExternalInput").ap()
    v_d = nc.dram_tensor("v", [B_LOC, S, VD], f32, kind="ExternalInput").ap()
    wq_d = nc.dram_tensor("Wq", [QD, H * KD], f32, kind="ExternalInput").ap()
    wk_d = nc.dram_tensor("Wk", [KD, H * KD], f32, kind="ExternalInput").ap()
    wv_d = nc.dram_tensor("Wv", [VD, H * VD], f32, kind="ExternalInput").ap()
    wc_d = nc.dram_tensor("Wc", [1024, 128], f32, kind="ExternalInput").ap()
    bq_d = nc.dram_tensor("bq", [1024], f32, kind="ExternalInput").ap()
    bk_d = nc.dram_tensor("bk", [1024], f32, kind="ExternalInput").ap()
    bv_d = nc.dram_tensor("bv", [1024], f32, kind="ExternalInput").ap()
    bc_d = nc.dram_tensor("bc", [128], f32, kind="ExternalInput").ap()
    eye_d = nc.dram_tensor("eye", [128, 128], f32, kind="ExternalInput").ap()
    m01_d = nc.dram_tensor("mask01", [128, 4], f32, kind="ExternalInput").ap()

    attn_d = nc.dram_tensor("attn", [B_LOC, H, S, S], f32,
                            kind="ExternalOutput").ap()
    out_d = nc.dram_tensor("out", [B_LOC, 128], f32,
                           kind="ExternalOutput").ap()

    with tile.TileContext(nc) as tc, ExitStack() as ctx:
        const = ctx.enter_context(tc.tile_pool(name="const", bufs=1))
        nat = ctx.enter_context(tc.tile_pool(name="nat", bufs=1))
        trp = ctx.enter_context(tc.tile_pool(name="trp", bufs=1))
        proj = ctx.enter_context(tc.tile_pool(name="proj", bufs=1))
        small = ctx.enter_context(tc.tile_pool(name="small", bufs=2))
        stat = ctx.enter_context(tc.tile_pool(name="stat", bufs=4))
        expp = ctx.enter_context(tc.tile_pool(name="expp", bufs=2))
        attp = ctx.enter_context(tc.tile_pool(name="attp", bufs=3))

        ps_sc = ctx.enter_context(
            tc.tile_pool(name="ps_sc", bufs=2, space="PSUM"))
        ps_r = ctx.enter_context(
            tc.tile_pool(name="ps_r", bufs=1, space="PSUM"))
        ps_m = ctx.enter_context(
            tc.tile_pool(name="ps_m", bufs=2, space="PSUM"))

        # ---- constants / weights into SBUF -------------------------------
        # fp32r matmul operands must be *written* by a compute op with
        # float32r output dtype (the BIR verifier enforces this), so DMA
        # lands in transient f32 staging tiles and a DVE copy rounds into
        # the persistent f32r tiles.
        wq_s = const.tile([128, 1024], f32r, tag="wq")
        stg = nat.tile([128, 1024], f32, tag="qn")
        nc.sync.dma_start(stg[:], wq_d[:])
        nc.vector.tensor_copy(wq_s[:], stg[:])

        wk_s = const.tile([128, 2048], f32r, tag="wk")
        stg = nat.tile([128, 2048], f32, tag="kn")
        nc.sync.dma_start(stg[:].rearrange("p (c f) -> p c f", c=2),
                          wk_d.rearrange("(c p) f -> p c f", p=128))
        nc.vector.tensor_copy(wk_s[:], stg[:])

        wv_s = const.tile([128, 2048], f32r, tag="wv")
        stg = nat.tile([128, 2048], f32, tag="vn")
        nc.sync.dma_start(stg[:].rearrange("p (c f) -> p c f", c=2),
                          wv_d.rearrange("(c p) f -> p c f", p=128))
        nc.vector.tensor_copy(wv_s[:], stg[:])

        wc_s = const.tile([128, 1024], f32r, tag="wc")
        stg = nat.tile([128, 1024], f32, tag="qn")
        nc.sync.dma_start(stg[:].rearrange("p (t f) -> p t f", t=8),
                          wc_d.rearrange("(t p) f -> p t f", p=128))
        nc.vector.tensor_copy(wc_s[:], stg[:])

        bq_c = const.tile([128, 8], f32, tag="bqc")
        nc.sync.dma_start(bq_c[:], bq_d.rearrange("(t p) -> p t", p=128))
        bk_c = const.tile([128, 8], f32, tag="bkc")
        nc.sync.dma_start(bk_c[:], bk_d.rearrange("(t p) -> p t", p=128))
        eye_s = const.tile([128, 128], f32, tag="eye")
        nc.sync.dma_start(eye_s[:], eye_d[:])

        m01_s = const.tile([128, 4], f32r, tag="m01")
        stg4 = trp.tile([128, 4], f32, tag="stg4")
        nc.sync.dma_start(stg4[:], m01_d[:])
        nc.vector.tensor_copy(m01_s[:], stg4[:])

        bc_row = const.tile([1, 128], f32r, tag="bcr")
        stgbc = trp.tile([1, 128], f32, tag="stgbc")
        nc.sync.dma_start(stgbc[0:1, :], bc_d[:])
        nc.vector.tensor_copy(bc_row[:], stgbc[:])

        bv_row = const.tile([1, 1024], f32r, tag="bvr")
        stgbv = trp.tile([1, 1024], f32, tag="stgbv")
        nc.sync.dma_start(stgbv[0:1, :], bv_d[:])
        nc.vector.tensor_copy(bv_row[:], stgbv[:])

        ones_s = const.tile([1, 128], f32r, tag="ones")
        stg1 = trp.tile([1, 128], f32, tag="stgbc")
        nc.vector.memset(stg1[:], 1.0)
        nc.vector.tensor_copy(ones_s[:], stg1[:])

        for b in range(B_LOC):
            # ---- load + transpose inputs ---------------------------------
            qn = nat.tile([128, 1024], f32, tag="qn")
            nc.sync.dma_start(qn[:].rearrange("p (t d) -> p t d", t=8),
                              q_d[b].rearrange("(t p) d -> p t d", p=128))
            kn = nat.tile([128, 2048], f32, tag="kn")
            nc.sync.dma_start(kn[:].rearrange("p (t d) -> p t d", t=8),
                              k_d[b].rearrange("(t p) d -> p t d", p=128))
            vn = nat.tile([128, 2048], f32, tag="vn")
            nc.sync.dma_start(vn[:].rearrange("p (t d) -> p t d", t=8),
                              v_d[b].rearrange("(t p) d -> p t d", p=128))

            qT = trp.tile([128, 1024], f32r, tag="qT")
            kT = trp.tile([128, 2, 1024], f32r, tag="kT")
            vT = trp.tile([128, 2, 1024], f32r, tag="vT")
            for t in range(8):
                ps = ps_m.tile([128, 128], f32, tag="misc")
                nc.tensor.transpose(ps[:], qn[:, t * 128:(t + 1) * 128],
                                    eye_s[:])
                nc.any.tensor_copy(qT[:, t * 128:(t + 1) * 128], ps[:])
            for t in range(8):
                for c in range(2):
                    ps = ps_m.tile([128, 128], f32, tag="misc")
                    nc.tensor.transpose(
                        ps[:], kn[:, t * 256 + c * 128:t * 256 + (c + 1) * 128],
                        eye_s[:])
                    nc.any.tensor_copy(kT[:, c, t * 128:(t + 1) * 128], ps[:])
            for t in range(8):
                for c in range(2):
                    ps = ps_m.tile([128, 128], f32, tag="misc")
                    nc.tensor.transpose(
                        ps[:], vn[:, t * 256 + c * 128:t * 256 + (c + 1) * 128],
                        eye_s[:])
                    nc.any.tensor_copy(vT[:, c, t * 128:(t + 1) * 128], ps[:])

            # ---- projections ---------------------------------------------
            # tqT[f, s], tkT[f, s] (transposed form, f on partitions);
            # tv[s, f] (natural form, s on partitions)
            # head-major layout: tqT[p, c, h, q'] = tq_hT[d = c*128+p, q']
            # where q' = qq*4 + j, f = j*256 + c*128 + p, s = h*256 + qq.
            tqT = proj.tile([128, 2, 4, 1024], f32r, tag="tqT")
            tkT = proj.tile([128, 2, 4, 1024], f32r, tag="tkT")
            tv = proj.tile([128, 8, 1024], f32r, tag="tv")

            def evac_bias(dst, ps, bias_ap, idx):
                # PSUM -> SBUF copy + per-partition bias add, alternating
                # engines to balance DVE/ACT load.
                if idx % 2 == 0:
                    nc.vector.tensor_scalar_add(dst, ps, bias_ap)
                else:
                    nc.scalar.activation(dst, ps, AF.Identity, bias=bias_ap)

            # evac scatters the [f-tile, s-chunk] psum tile into the
            # head-major layout: m -> (j = m//2, c = m%2), s-chunk covers
            # two heads (hh outer, qq inner order matches psum order).
            tqT_w = tqT[:].rearrange("p c hh (qq j) -> p c hh qq j", j=4)
            tkT_w = tkT[:].rearrange("p c hh (qq j) -> p c hh qq j", j=4)
            idx = 0
            for m in range(8):
                j, c = m // 2, m % 2
                for sc in range(2):
                    ps = ps_m.tile([128, 512], f32, tag="misc")
                    nc.tensor.matmul(
                        ps[:], wq_s[:, m * 128:(m + 1) * 128],
                        qT[:, sc * 512:(sc + 1) * 512],
                        start=True, stop=True)
                    evac_bias(tqT_w[:, c, sc * 2:(sc + 1) * 2, :, j], ps[:],
                              bq_c[:, m:m + 1], idx)
                    idx += 1
            for m in range(8):
                j, c2 = m // 2, m % 2
                for sc in range(2):
                    ps = ps_m.tile([128, 512], f32, tag="misc")
                    for c in range(2):
                        nc.tensor.matmul(
                            ps[:],
                            wk_s[:, c * 1024 + m * 128:
                                 c * 1024 + (m + 1) * 128],
                            kT[:, c, sc * 512:(sc + 1) * 512],
                            start=(c == 0), stop=(c == 1))
                    evac_bias(tkT_w[:, c2, sc * 2:(sc + 1) * 2, :, j], ps[:],
                              bk_c[:, m:m + 1], idx)
                    idx += 1
            for st in range(8):
                for fc in range(2):
                    ps = ps_m.tile([128, 512], f32, tag="misc")
                    for c in range(2):
                        nc.tensor.matmul(
                            ps[:],
                            vT[:, c, st * 128:(st + 1) * 128],
                            wv_s[:, c * 1024 + fc * 512:
                                 c * 1024 + fc * 512 + 512],
                            start=(c == 0), stop=False)
                    # + bv broadcast along partitions via rank-1 accumulate
                    nc.tensor.matmul(
                        ps[:], ones_s[0:1, :],
                        bv_row[0:1, fc * 512:(fc + 1) * 512],
                        start=False, stop=True)
                    nc.any.tensor_copy(tv[:, st, fc * 512:(fc + 1) * 512],
                                       ps[:])

            # ---- attention per head --------------------------------------
            Z_sb = small.tile([4, 256], f32, tag="Z")
            for h in range(H):
                R_ps = ps_r.tile([4, 1024], f32, tag="R")
                for qt in range(8):
                    sc_ps = ps_sc.tile([128, 1024], f32, tag="sc")
                    for c in range(2):
                        lhs = tqT[:, c, h, qt * 128:(qt + 1) * 128]
                        for nch in range(2):
                            rhs = tkT[:, c, h, nch * 512:(nch + 1) * 512]
                            nc.tensor.matmul(
                                sc_ps[:, nch * 512:(nch + 1) * 512],
                                lhs, rhs,
                                start=(c == 0), stop=(c == 1))
                    # softmax (no max subtraction: |scores| is small)
                    exp_t = expp.tile([128, 1024], f32, tag="exp")
                    rowsum = stat.tile([128, 1], f32, tag="rs")
                    nc.scalar.activation(exp_t[:], sc_ps[:], AF.Exp,
                                         accum_out=rowsum[:])
                    recip = stat.tile([128, 1], f32, tag="rc")
                    nc.vector.reciprocal(recip[:], rowsum[:])
                    attn_t = attp.tile([128, 1024], f32r, tag="attn")
                    nc.vector.tensor_scalar_mul(attn_t[:], exp_t[:], recip[:])
                    nc.sync.dma_start(
                        attn_d[b, h, qt * 128:(qt + 1) * 128, :],
                        attn_t[:].bitcast(f32))
                    # R[j,k] += sum_{q'=j mod 4} attn[q',k]
                    for nch in range(2):
                        nc.tensor.matmul(
                            R_ps[0:4, nch * 512:(nch + 1) * 512],
                            m01_s[:],
                            attn_t[:, nch * 512:(nch + 1) * 512],
                            start=(qt == 0), stop=(qt == 7))

                # Z[j,:] += sum_{k} R[j,k] tv_h[k,:] for this head
                R_sb = small.tile([4, 1024], f32, tag="Rsb")
                nc.any.tensor_copy(R_sb[:], R_ps[:])
                R_r = R_sb[:].rearrange("p (kk i) -> p i kk", i=4)
                RT = small.tile([128, 8, 4], f32r, tag="RT")
                for i in range(4):
                    for kc in range(2):
                        ps = ps_m.tile([128, 128], f32, tag="misc")
                        nc.tensor.transpose(
                            ps[0:128, 0:4],
                            R_r[0:4, i, kc * 128:(kc + 1) * 128],
                            eye_s[0:4, 0:4])
                        nc.any.tensor_copy(RT[:, i * 2 + kc, :],
                                           ps[0:128, 0:4])
                Z_ps = ps_m.tile([128, 512], f32, tag="misc")
                n = 0
                for i in range(4):
                    for kc in range(2):
                        nc.tensor.matmul(
                            Z_ps[0:4, 0:256],
                            RT[:, i * 2 + kc, :],
                            tv[:, 2 * h + kc,
                               i * 256:(i + 1) * 256],
                            start=(n == 0), stop=(n == 7))
                        n += 1
                if h == 0:
                    nc.vector.tensor_copy(Z_sb[:], Z_ps[0:4, 0:256])
                else:
                    nc.vector.tensor_add(Z_sb[:], Z_sb[:], Z_ps[0:4, 0:256])

            # ---- out head: out[b] = (Z.flat/1024) @ (Wo@W1) + bc ---------
            # (the 1/1024 is folded into Wc on the host)
            # attmean[256*j + half*128 + p] = Z[j, half*128 + p] = zT[half][p, j]
            zT = small.tile([128, 2, 4], f32r, tag="zT")
            for half in range(2):
                ps = ps_m.tile([128, 128], f32, tag="misc")
                nc.tensor.transpose(
                    ps[0:128, 0:4],
                    Z_sb[0:4, half * 128:(half + 1) * 128],
                    eye_s[0:4, 0:4])
                nc.any.tensor_copy(zT[:, half, :], ps[0:128, 0:4])
            out_ps = ps_m.tile([128, 512], f32, tag="misc")
            for t in range(8):
                j, half = t // 2, t % 2
                nc.tensor.matmul(
                    out_ps[0:1, 0:128],
                    zT[:, half, j:j + 1],
                    wc_s[:, t * 128:(t + 1) * 128],
                    start=(t == 0), stop=False)
            nc.tensor.matmul(out_ps[0:1, 0:128], ones_s[0:1, 0:1],
                             bc_row[0:1, :], start=False, stop=True)
            out_sb = small.tile([1, 128], f32, tag="outsb")
            nc.any.tensor_copy(out_sb[:], out_ps[0:1, 0:128])
            nc.sync.dma_start(out_d[b:b + 1, :], out_sb[:])

    nc.compile()
    return nc


def _get_nc():
    if "nc" not in _CACHE:
        _CACHE["nc"] = _build_nc()
    return _CACHE["nc"]


def _make_in_maps(inputs):
    q = np.ascontiguousarray(np.asarray(inputs["query"], dtype=np.float32))
    k = np.ascontiguousarray(np.asarray(inputs["key"], dtype=np.float32))
    v = np.ascontiguousarray(np.asarray(inputs["value"], dtype=np.float32))
    Wq = np.asarray(inputs["Wq"], dtype=np.float32)
    Wk = np.asarray(inputs["Wk"], dtype=np.float32)
    Wv = np.asarray(inputs["Wv"], dtype=np.float32)
    bq = np.asarray(inputs["bq"], dtype=np.float32)
    bk = np.asarray(inputs["bk"], dtype=np.float32)
    bv = np.asarray(inputs["bv"], dtype=np.float32)
    Wo = np.asarray(inputs["Wo"], dtype=np.float64)
    bo = np.asarray(inputs["bo"], dtype=np.float64)
    W1 = np.asarray(inputs["W1"], dtype=np.float64)
    b1 = np.asarray(inputs["b1"], dtype=np.float64)

    Wc = ((Wo @ W1) / 1024.0).astype(np.float32)
    bc = (bo @ W1 + b1).astype(np.float32)
    eye = np.eye(128, dtype=np.float32)
    mask01 = np.zeros((128, 4), dtype=np.float32)
    mask01[np.arange(128), np.arange(128) % 4] = 1.0

    shared = {"Wq": Wq, "Wk": Wk, "Wv": Wv, "Wc": Wc, "bq": bq, "bk": bk,
              "bv": bv, "bc": bc, "eye": eye, "mask01": mask01}
    in_maps = []
    for c in range(N_CORES):
        sl = slice(c * B_LOC, (c + 1) * B_LOC)
        in_maps.append({"q": q[sl], "k": k[sl], "v": v[sl], **shared})
    return in_maps


def _run(inputs, trace=False, **kw):
    from concourse.bass_utils import run_bass_kernel_spmd

    nc = _get_nc()
    in_maps = _make_in_maps(inputs)
    res = run_bass_kernel_spmd(nc, in_maps, core_ids=list(range(N_CORES)),
                               trace=trace, **kw)
    attn = np.empty((B, H, S, S), dtype=np.float32)
    out = np.empty((B, 128), dtype=np.float32)
    for c in range(N_CORES):
        sl = slice(c * B_LOC, (c + 1) * B_LOC)
        attn[sl] = res.results[c]["attn"]
        out[sl] = res.results[c]["out"]
    return (out, attn), res


def kernel(**inputs):
    (out, attn), _ = _run(inputs)
    return out, attn
